# revision 24
# baseline (speedup 1.0000x reference)
"""DTCWT forward (J=3) Trainium2 Bass kernel.

Strategy: pure data parallel over the 256 (b, c) channels, 32 per NeuronCore.
Each channel's whole 3-level transform is expressed as two matmul stages with
host-precomputed composite operator matrices (reflection padding, decimation
phases, q2c parity subsampling, butterfly and 1/sqrt2 scale all folded in):

  stage 1 (contract over H):  xc = x^T @ U_all                [256, 1024]
  stage 2 (contract over W):  pair_block = sum_par xc[:, Upar]^T @ Vpar

v2: first-pass-full / later-passes-band-limited PSUM accumulation,
channel-pair packing for level 2, channel-quad packing for level 3 and ll3,
merged lh+hh matmuls at levels 2/3, batched DMAs split across both HWDGE
rings (sync + scalar).
"""

import sys

import numpy as np

for _p in ("/opt/trn_rl_repo", "/root/.axon_site/_ro/trn_rl_repo"):
    try:
        import concourse  # noqa: F401
        break
    except ImportError:
        if _p not in sys.path:
            sys.path.append(_p)

_F64 = np.float64
_S2 = 1.0 / np.sqrt(2.0)

# matmul operand dtype: "float32r" (1 cyc/row, reduced precision on HW) or
# "float32" (4 cyc/row, full precision)
MM_DTYPE = "float32r"

N_CORES = 8
CH_PER_CORE = 32

# ---------------------------------------------------------------------------
# host-side operator construction (exact math, float64 -> float32)
# ---------------------------------------------------------------------------


def _reflect(x, minx, maxx):
    rng = maxx - minx
    rng2 = 2.0 * rng
    mod = np.fmod(x - minx, rng2)
    mod = np.where(mod < 0, mod + rng2, mod)
    out = np.where(mod >= rng, rng2 - mod, mod) + minx
    return out.astype(np.int64)


def _symm_pad(l, m):
    return _reflect(np.arange(-m, l + m, dtype=np.float64), -0.5, l - 0.5)


def _conv_mat(h, L):
    """M [L, L] with rowfilter(X, h) == X @ M."""
    h = np.asarray(h, dtype=_F64)
    m = len(h) // 2
    xe = _symm_pad(L, m)
    M = np.zeros((L, L), dtype=_F64)
    for j in range(L):
        for k in range(len(h)):
            M[xe[j + k], j] += h[k]
    return M


def _dec_mat(ha, hb, highpass, L):
    """D [L, L//2] with rowdfilt(X, ha, hb, highpass) == X @ D."""
    ha = np.asarray(ha, dtype=_F64)
    hb = np.asarray(hb, dtype=_F64)
    m = len(ha)
    xe = _symm_pad(L, m)
    sel_a = xe[2::2]
    sel_b = xe[3::2]
    ya = np.zeros((L, L // 4), dtype=_F64)
    yb = np.zeros((L, L // 4), dtype=_F64)
    for t in range(L // 4):
        for k in range(m):
            ya[sel_a[2 * t + k], t] += ha[k]
            yb[sel_b[2 * t + k], t] += hb[k]
    if highpass:
        ya, yb = yb, ya
    D = np.zeros((L, L // 2), dtype=_F64)
    D[:, 0::2] = ya
    D[:, 1::2] = yb
    return D


def _interleave_cols(A, B):
    out = np.zeros((A.shape[0], 2 * A.shape[1]), dtype=A.dtype)
    out[:, 0::2] = A
    out[:, 1::2] = B
    return out


def _pair_rhs(V, parity):
    """Stage-2 rhs [256, 4n] for an orientation-pair block.
    cols = [A_r/A_i interleaved | B_r/B_i interleaved];
    A = s(a-d), A_i = s(b+c), B = s(a+d), B_i = s(b-c)."""
    Ve = V[:, 0::2] * _S2
    Vo = V[:, 1::2] * _S2
    if parity == "e":
        half = _interleave_cols(Ve, Vo)
        return np.concatenate([half, half], axis=1)
    return np.concatenate(
        [_interleave_cols(-Vo, Ve), _interleave_cols(Vo, -Ve)], axis=1
    )


# static layout: U_all column blocks (offset, ncols), padded to 1024 so the
# two 512-col halves each fill exactly one PSUM bank per matmul.
_UBLOCKS = {
    "C0e": (0, 128), "C0o": (128, 128), "C1e": (256, 128), "C1o": (384, 128),
    "PE0e": (512, 64), "PE0o": (576, 64), "PE1e": (640, 64), "PE1o": (704, 64),
    "PF0e": (768, 32), "PF0o": (800, 32), "PF1e": (832, 32), "PF1o": (864, 32),
}
_NU = 1024

# stage-2 matmul groups.
#   pack: channels sharing one matmul (1 = per-channel, 2 = pair, 4 = quad)
#   padn: streamed N (zero-padded); realn: evacuated/real cols
_GROUPS = [
    # (name, upre, level, opairs, realn, padn, pack)
    ("lh1", "C1", 1, [(0, 5)], 512, 512, 1),
    ("hh1", "C1", 1, [(1, 4)], 512, 512, 1),
    ("hl1", "C0", 1, [(2, 3)], 512, 512, 1),
    ("lhh2", "PE1", 2, [(0, 5), (1, 4)], 512, 512, 2),
    ("hl2", "PE0", 2, [(2, 3)], 256, 256, 2),
    ("lhh3", "PF1", 3, [(0, 5), (1, 4)], 256, 256, 4),
    ("hl3", "PF0", 3, [(2, 3)], 128, 256, 4),
]


def _group_rhs(gname, consts_v, par):
    """full [256, padn] rhs (float64) for a group/parity."""
    for g in _GROUPS:
        if g[0] == gname:
            _, _, lvl, opairs, realn, padn, _ = g
            break
    vs = consts_v[gname]
    parts = [_pair_rhs(V, par) for V in vs]
    rhs = np.concatenate(parts, axis=1)
    full = np.zeros((256, padn), dtype=_F64)
    full[:, :rhs.shape[1]] = rhs
    return full


def _make_vsrc(h0o, h1o, h0a, h0b, h1a, h1b):
    R0 = _conv_mat(h0o, 256)
    R1 = _conv_mat(h1o, 256)
    C0, C1 = R0, R1
    D0 = _dec_mat(h0b, h0a, False, 256)
    D1 = _dec_mat(h1b, h1a, True, 256)
    E0, E1 = D0, D1
    G0 = _dec_mat(h0b, h0a, False, 128)
    G1 = _dec_mat(h1b, h1a, True, 128)
    F0, F1 = G0, G1

    PE0 = C0 @ E0
    PE1 = C0 @ E1
    PE0F0 = PE0 @ F0
    PE0F1 = PE0 @ F1
    QD0 = R0 @ D0
    QD1 = R0 @ D1
    QD0G0 = QD0 @ G0
    QD0G1 = QD0 @ G1

    ucols = {
        "C0e": C0[:, 0::2], "C0o": C0[:, 1::2],
        "C1e": C1[:, 0::2], "C1o": C1[:, 1::2],
        "PE0e": PE0[:, 0::2], "PE0o": PE0[:, 1::2],
        "PE1e": PE1[:, 0::2], "PE1o": PE1[:, 1::2],
        "PF0e": PE0F0[:, 0::2], "PF0o": PE0F0[:, 1::2],
        "PF1e": PE0F1[:, 0::2], "PF1o": PE0F1[:, 1::2],
    }
    # V matrices per group (list => concatenated side by side)
    gv = {"lh1": [R0], "hh1": [R1], "hl1": [R1],
          "lhh2": [QD0, QD1], "hl2": [QD1],
          "lhh3": [QD0G0, QD0G1], "hl3": [QD0G1]}
    return ucols, gv, QD0G0


def _build_consts(h0o, h1o, h0a, h0b, h1a, h1b):
    ucols, gv, QD0G0 = _make_vsrc(h0o, h1o, h0a, h0b, h1a, h1b)

    U_all = np.zeros((256, _NU), dtype=_F64)
    for name, (off, nc_) in _UBLOCKS.items():
        M = ucols[name]
        assert M.shape == (256, nc_)
        U_all[:, off:off + nc_] = M

    blocks = [("u", U_all)]
    for g in _GROUPS:
        gname = g[0]
        for par in ("e", "o"):
            blocks.append((f"{gname}{par}", _group_rhs(gname, gv, par)))
    blocks.append(("ll3v", QD0G0))
    cc = np.concatenate([b[1] for b in blocks], axis=1)
    return {f"cc{k}": np.ascontiguousarray(
        cc[k * 128:(k + 1) * 128]).astype(np.float32) for k in range(2)}


def _const_offsets():
    """column offsets of each packed constant block inside cc{k}."""
    offs = {}
    off = 0
    for name, w in ([("u", _NU)]
                    + [(f"{g[0]}{par}", g[5]) for g in _GROUPS
                       for par in ("e", "o")]
                    + [("ll3v", 64)]):
        offs[name] = off
        off += w
    return offs, off


def _build_ranges():
    """Structural nonzero column ranges per (group, parity, kchunk), computed
    with all-ones filters (depends only on tap counts, not values).
    Pass 1 (e, k=0) is always streamed full-N with start=True."""
    ucols, gv, _ = _make_vsrc(np.ones(5), np.ones(7), np.ones(10),
                              np.ones(10), np.ones(10), np.ones(10))
    ranges = {}
    for g in _GROUPS:
        gname, padn = g[0], g[5]
        cost_full = padn * (1 if padn >= 256 else 4)
        for par in ("e", "o"):
            full = _group_rhs(gname, gv, par)
            for k in range(2):
                blk = full[k * 128:(k + 1) * 128]
                nz = np.nonzero(np.any(blk != 0.0, axis=0))[0]
                lo, hi = int(nz.min()), int(nz.max()) + 1
                w = hi - lo
                # float32r runs 4 cyc/row below N=256 — band-limiting only
                # pays off when the narrowed stream is still >=256 wide
                cost_rng = w * (1 if w >= 256 else 4)
                if cost_rng >= cost_full:
                    lo, hi = 0, padn
                ranges[(gname, par, k)] = (lo, hi)
    return ranges



# xc group-tile layout (4 channels, block-major): each U-block's 4 channel
# copies are adjacent so channel-packed lhsT slices are contiguous 1-D APs.
_XCOFF = {
    "C0e": (0, 128), "C0o": (512, 128), "C1e": (1024, 128), "C1o": (1536, 128),
    "PE0e": (2048, 64), "PE0o": (2304, 64), "PE1e": (2560, 64),
    "PE1o": (2816, 64),
    "PF0e": (3072, 32), "PF0o": (3200, 32), "PF1e": (3328, 32),
    "PF1o": (3456, 32),
}

# ---------------------------------------------------------------------------
# bass program (built once; constants are runtime inputs)
# ---------------------------------------------------------------------------

_CACHE = {}


def _build_program():
    import concourse.tile as tile
    from concourse import bacc, mybir
    from contextlib import ExitStack

    mmdt = getattr(mybir.dt, MM_DTYPE)
    f32 = mybir.dt.float32
    ranges = _build_ranges()

    nc = bacc.Bacc("TRN2", target_bir_lowering=False, debug=False)

    x_d = nc.dram_tensor("x", [CH_PER_CORE, 256, 256], mmdt,
                         kind="ExternalInput").ap()
    coffs, ctot = _const_offsets()
    const_d = {f"cc{k}": nc.dram_tensor(f"cc{k}", [128, ctot], mmdt,
                                        kind="ExternalInput").ap()
               for k in range(2)}

    yl_d = nc.dram_tensor("yl", [CH_PER_CORE, 64, 64], f32,
                          kind="ExternalOutput").ap()
    yh1_d = nc.dram_tensor("yh1", [CH_PER_CORE, 6, 128, 128, 2], f32,
                           kind="ExternalOutput").ap()
    # orientation-major so (channel, h) fuse into the DMA partition dim
    yh2_d = nc.dram_tensor("yh2", [6, CH_PER_CORE, 64, 64, 2], f32,
                           kind="ExternalOutput").ap()
    yh3_d = nc.dram_tensor("yh3", [6, CH_PER_CORE, 32, 32, 2], f32,
                           kind="ExternalOutput").ap()
    yh_d = {1: yh1_d, 2: yh2_d, 3: yh3_d}

    copy_ctr = [0]
    dma_ctr = [0]

    with tile.TileContext(nc) as tc:
        with ExitStack() as ctx:
            cpool = ctx.enter_context(tc.tile_pool(name="consts", bufs=1))
            xpool = ctx.enter_context(tc.tile_pool(name="xin", bufs=8))
            xcps_pool = ctx.enter_context(
                tc.tile_pool(name="xcps", bufs=4, space="PSUM"))
            xc_pool = ctx.enter_context(tc.tile_pool(name="xcsb", bufs=2))
            ps2_pool = ctx.enter_context(
                tc.tile_pool(name="ps2", bufs=4, space="PSUM"))
            stg_pool = ctx.enter_context(tc.tile_pool(name="stg", bufs=3))

            def copy_out(dst, src):
                if copy_ctr[0] % 2 == 0:
                    nc.vector.tensor_copy(dst, src)
                else:
                    nc.scalar.copy(dst, src)
                copy_ctr[0] += 1

            def dma(dst, src):
                if dma_ctr[0] % 2 == 0:
                    nc.sync.dma_start(dst, src)
                else:
                    nc.scalar.dma_start(dst, src)
                dma_ctr[0] += 1

            ccsb = []
            for k in range(2):
                t = cpool.tile([128, ctot], mmdt, tag=f"cc{k}",
                               name=f"cc{k}_sb")
                dma(t[:], const_d[f"cc{k}"][:])
                ccsb.append(t)

            def cslice(name, k, lo, hi):
                return ccsb[k][:, coffs[name] + lo:coffs[name] + hi]

            def phase_a(g0):
                """x load + stage 1 for the 4 channels of group g0."""
                xcg = [xc_pool.tile([128, 4 * _NU], mmdt, tag=f"xcg{m}",
                                    name=f"xcg{m}_{g0}")
                       for m in range(2)]
                for ci in range(4):
                    ch = g0 + ci
                    xin = xpool.tile([128, 512], mmdt, tag="xin",
                                     name=f"xin_{ch}")
                    # partition p holds x[ch, p, :] | x[ch, p+128, :]
                    dma(xin[:].rearrange("p (k w) -> p k w", k=2),
                        x_d[ch].rearrange("(k p) w -> p k w", k=2))
                    for m in range(2):
                        for half in range(2):
                            ps = xcps_pool.tile([128, 512], f32, tag="xcps")
                            for k in range(2):
                                nc.tensor.matmul(
                                    ps[:, :],
                                    lhsT=xin[:, k * 256 + m * 128:
                                             k * 256 + (m + 1) * 128],
                                    rhs=cslice("u", k, half * 512,
                                               (half + 1) * 512),
                                    start=(k == 0), stop=(k == 1))
                            if half == 0:
                                # 4 L1 blocks of 128 -> block-major scatter
                                dst = xcg[m][:, 0:2048].rearrange(
                                    "p (b c j) -> p b c j", b=4, c=4)
                                copy_out(dst[:, :, ci, :], ps[:, :])
                            else:
                                # 4 blocks of 64, then 4 blocks of 32
                                dst = xcg[m][:, 2048:3072].rearrange(
                                    "p (b c j) -> p b c j", b=4, c=4)
                                copy_out(dst[:, :, ci, :], ps[:, 0:256])
                                dst = xcg[m][:, 3072:3584].rearrange(
                                    "p (b c j) -> p b c j", b=4, c=4)
                                copy_out(dst[:, :, ci, :], ps[:, 256:384])
                return xcg

            def phase_b(g0, xcg):
                def s2_matmuls(gname, upre, ps, ci0, pack, padn):
                    """4 accumulation passes (par x kchunk) into ps."""
                    first = True
                    for par in ("e", "o"):
                        for k in range(2):
                            if first:
                                lo, hi = 0, padn
                            else:
                                lo, hi = ranges[(gname, par, k)]
                                # PSUM write offsets must be 128-col aligned
                                lo = (lo // 128) * 128
                                w = hi - lo
                                if w * (1 if w >= 256 else 4) >= padn:
                                    lo, hi = 0, padn
                            base, w = _XCOFF[upre + par]
                            lhsT = xcg[k][:, base + ci0 * w:
                                          base + (ci0 + pack) * w]
                            nc.tensor.matmul(
                                ps[:, lo:hi],
                                lhsT=lhsT,
                                rhs=cslice(gname + par, k, lo, hi),
                                start=first,
                                stop=(par == "o" and k == 1))
                            first = False

                # ---- level 1: per channel ----
                for ci in range(4):
                    ch = g0 + ci
                    for gname, upre, lvl, opairs, realn, padn, pack in \
                            _GROUPS[:3]:
                        ps = ps2_pool.tile([128, padn], f32, tag="ps2")
                        s2_matmuls(gname, upre, ps, ci, 1, padn)
                        stg = stg_pool.tile(
                            [128, 512], f32, tag=f"stg_{gname}",
                            name=f"stg_{gname}_{ch}")
                        copy_out(stg[:, :], ps[:, :])
                        oA, oB = opairs[0]
                        dest = yh_d[1][ch, oA:oB + 1:oB - oA].rearrange(
                            "o h w r -> h o (w r)")
                        dma(dest,
                            stg[:].rearrange("p (o n) -> p o n", o=2))

                # ---- level 2: per channel pair ----
                for ci0 in (0, 2):
                    ch0 = g0 + ci0
                    for gname, upre, lvl, opairs, realn, padn, pack in \
                            _GROUPS[3:5]:
                        ps = ps2_pool.tile([128, padn], f32, tag="ps2")
                        s2_matmuls(gname, upre, ps, ci0, 2, padn)
                        stg = stg_pool.tile([128, realn], f32,
                                            tag=f"stg_{gname}")
                        copy_out(stg[:, :], ps[:, 0:realn])
                        npair = realn // len(opairs)
                        for pi, (oA, oB) in enumerate(opairs):
                            dest = yh_d[2][oA:oB + 1:oB - oA,
                                           ch0:ch0 + 2].rearrange(
                                "o c h w r -> (c h) o (w r)")
                            dma(dest,
                                stg[:, pi * npair:(pi + 1) * npair]
                                .rearrange("p (o n) -> p o n", o=2))

                # ---- level 3 + ll3: per channel quad ----
                for gname, upre, lvl, opairs, realn, padn, pack in \
                        _GROUPS[5:]:
                    ps = ps2_pool.tile([128, padn], f32, tag="ps2")
                    s2_matmuls(gname, upre, ps, 0, 4, padn)
                    stg = stg_pool.tile([128, realn], f32, tag=f"stg_{gname}")
                    copy_out(stg[:, :], ps[:, 0:realn])
                    npair = realn // len(opairs)
                    for pi, (oA, oB) in enumerate(opairs):
                        dest = yh_d[3][oA:oB + 1:oB - oA,
                                       g0:g0 + 4].rearrange(
                            "o c h w r -> (c h) o (w r)")
                        dma(dest,
                            stg[:, pi * npair:(pi + 1) * npair]
                            .rearrange("p (o n) -> p o n", o=2))

                for par in ("e", "o"):
                    base, w = _XCOFF["PF0" + par]
                    ps = ps2_pool.tile([128, 64], f32, tag="ps2")
                    for k in range(2):
                        nc.tensor.matmul(
                            ps[:, :],
                            lhsT=xcg[k][:, base:base + 4 * w],
                            rhs=cslice("ll3v", k, 0, 64),
                            start=(k == 0), stop=(k == 1))
                    stg = stg_pool.tile([128, 64], f32, tag="stgll")
                    copy_out(stg[:, :], ps[:, :])
                    p0 = 0 if par == "e" else 1
                    dest = yl_d[g0:g0 + 4, p0::2, :].rearrange(
                        "c h w -> (c h) w")
                    dma(dest, stg[:, :])

            # software-pipelined emission: stage-1 of group g+1 is issued
            # to the PE before stage-2 of group g, so the PE never idles
            # (and HAM never re-throttles) at group boundaries
            groups = list(range(0, CH_PER_CORE, 4))
            pend = []  # [(g0, xcg)]
            for g0 in groups:
                pend.append((g0, phase_a(g0)))
                if len(pend) == 2:
                    pg0, pxcg = pend.pop(0)
                    phase_b(pg0, pxcg)
            for pg0, pxcg in pend:
                phase_b(pg0, pxcg)

    nc.compile()
    return nc


def _get_program():
    if "nc" not in _CACHE:
        _CACHE["nc"] = _build_program()
    return _CACHE["nc"]


# ---------------------------------------------------------------------------
# public entry point
# ---------------------------------------------------------------------------

def kernel(x, h0o, h1o, h0a, h0b, h1a, h1b, _return_results=False,
           _trace=False):
    from concourse import bass_utils

    x = np.ascontiguousarray(np.asarray(x, dtype=np.float32))
    B, C, H, W = x.shape
    assert (B, C, H, W) == (4, 64, 256, 256)

    consts = _build_consts(
        np.asarray(h0o, np.float64), np.asarray(h1o, np.float64),
        np.asarray(h0a, np.float64), np.asarray(h0b, np.float64),
        np.asarray(h1a, np.float64), np.asarray(h1b, np.float64))

    nc = _get_program()

    xf = x.reshape(B * C, H, W)
    in_maps = []
    for core in range(N_CORES):
        m = {"x": np.ascontiguousarray(
            xf[core * CH_PER_CORE:(core + 1) * CH_PER_CORE])}
        m.update(consts)
        in_maps.append(m)

    r = bass_utils.run_bass_kernel_spmd(nc, in_maps,
                                        core_ids=list(range(N_CORES)),
                                        trace=_trace)

    yl = np.concatenate([r.results[i]["yl"] for i in range(N_CORES)], axis=0)
    yh1 = np.concatenate([r.results[i]["yh1"] for i in range(N_CORES)], axis=0)
    yh2 = np.concatenate([r.results[i]["yh2"] for i in range(N_CORES)],
                         axis=1).transpose(1, 0, 2, 3, 4)
    yh3 = np.concatenate([r.results[i]["yh3"] for i in range(N_CORES)],
                         axis=1).transpose(1, 0, 2, 3, 4)

    out = (yl.reshape(B, C, 64, 64),
           yh1.reshape(B, C, 6, 128, 128, 2),
           np.ascontiguousarray(yh2).reshape(B, C, 6, 64, 64, 2),
           np.ascontiguousarray(yh3).reshape(B, C, 6, 32, 32, 2))
    if _return_results:
        return out, r
    return out


# revision 25
# speedup vs baseline: 1.0203x; 1.0203x over previous
"""DTCWT forward (J=3) Trainium2 Bass kernel.

Strategy: pure data parallel over the 256 (b, c) channels, 32 per NeuronCore.
Each channel's whole 3-level transform is expressed as two matmul stages with
host-precomputed composite operator matrices (reflection padding, decimation
phases, q2c parity subsampling, butterfly and 1/sqrt2 scale all folded in):

  stage 1 (contract over H):  xc = x^T @ U_all                [256, 1024]
  stage 2 (contract over W):  pair_block = sum_par xc[:, Upar]^T @ Vpar

v2: first-pass-full / later-passes-band-limited PSUM accumulation,
channel-pair packing for level 2, channel-quad packing for level 3 and ll3,
merged lh+hh matmuls at levels 2/3, batched DMAs split across both HWDGE
rings (sync + scalar).
"""

import sys

import numpy as np

for _p in ("/opt/trn_rl_repo", "/root/.axon_site/_ro/trn_rl_repo"):
    try:
        import concourse  # noqa: F401
        break
    except ImportError:
        if _p not in sys.path:
            sys.path.append(_p)

_F64 = np.float64
_S2 = 1.0 / np.sqrt(2.0)

# matmul operand dtype: "float32r" (1 cyc/row, reduced precision on HW) or
# "float32" (4 cyc/row, full precision)
MM_DTYPE = "float32r"

N_CORES = 8
CH_PER_CORE = 32

# ---------------------------------------------------------------------------
# host-side operator construction (exact math, float64 -> float32)
# ---------------------------------------------------------------------------


def _reflect(x, minx, maxx):
    rng = maxx - minx
    rng2 = 2.0 * rng
    mod = np.fmod(x - minx, rng2)
    mod = np.where(mod < 0, mod + rng2, mod)
    out = np.where(mod >= rng, rng2 - mod, mod) + minx
    return out.astype(np.int64)


def _symm_pad(l, m):
    return _reflect(np.arange(-m, l + m, dtype=np.float64), -0.5, l - 0.5)


def _conv_mat(h, L):
    """M [L, L] with rowfilter(X, h) == X @ M."""
    h = np.asarray(h, dtype=_F64)
    m = len(h) // 2
    xe = _symm_pad(L, m)
    M = np.zeros((L, L), dtype=_F64)
    for j in range(L):
        for k in range(len(h)):
            M[xe[j + k], j] += h[k]
    return M


def _dec_mat(ha, hb, highpass, L):
    """D [L, L//2] with rowdfilt(X, ha, hb, highpass) == X @ D."""
    ha = np.asarray(ha, dtype=_F64)
    hb = np.asarray(hb, dtype=_F64)
    m = len(ha)
    xe = _symm_pad(L, m)
    sel_a = xe[2::2]
    sel_b = xe[3::2]
    ya = np.zeros((L, L // 4), dtype=_F64)
    yb = np.zeros((L, L // 4), dtype=_F64)
    for t in range(L // 4):
        for k in range(m):
            ya[sel_a[2 * t + k], t] += ha[k]
            yb[sel_b[2 * t + k], t] += hb[k]
    if highpass:
        ya, yb = yb, ya
    D = np.zeros((L, L // 2), dtype=_F64)
    D[:, 0::2] = ya
    D[:, 1::2] = yb
    return D


def _interleave_cols(A, B):
    out = np.zeros((A.shape[0], 2 * A.shape[1]), dtype=A.dtype)
    out[:, 0::2] = A
    out[:, 1::2] = B
    return out


def _pair_rhs(V, parity):
    """Stage-2 rhs [256, 4n] for an orientation-pair block.
    cols = [A_r/A_i interleaved | B_r/B_i interleaved];
    A = s(a-d), A_i = s(b+c), B = s(a+d), B_i = s(b-c)."""
    Ve = V[:, 0::2] * _S2
    Vo = V[:, 1::2] * _S2
    if parity == "e":
        half = _interleave_cols(Ve, Vo)
        return np.concatenate([half, half], axis=1)
    return np.concatenate(
        [_interleave_cols(-Vo, Ve), _interleave_cols(Vo, -Ve)], axis=1
    )


# static layout: U_all column blocks (offset, ncols), padded to 1024 so the
# two 512-col halves each fill exactly one PSUM bank per matmul.
_UBLOCKS = {
    "C0e": (0, 128), "C0o": (128, 128), "C1e": (256, 128), "C1o": (384, 128),
    "PE0e": (512, 64), "PE0o": (576, 64), "PE1e": (640, 64), "PE1o": (704, 64),
    "PF0e": (768, 32), "PF0o": (800, 32), "PF1e": (832, 32), "PF1o": (864, 32),
}
_NU = 1024

# stage-2 matmul groups.
#   pack: channels sharing one matmul (1 = per-channel, 2 = pair, 4 = quad)
#   padn: streamed N (zero-padded); realn: evacuated/real cols
_GROUPS = [
    # (name, upre, level, opairs, realn, padn, pack)
    ("lh1", "C1", 1, [(0, 5)], 512, 512, 1),
    ("hh1", "C1", 1, [(1, 4)], 512, 512, 1),
    ("hl1", "C0", 1, [(2, 3)], 512, 512, 1),
    ("lhh2", "PE1", 2, [(0, 5), (1, 4)], 512, 512, 2),
    ("hl2", "PE0", 2, [(2, 3)], 256, 256, 2),
    ("lhh3", "PF1", 3, [(0, 5), (1, 4)], 256, 256, 4),
    ("hl3", "PF0", 3, [(2, 3)], 128, 256, 4),
]


def _group_rhs(gname, consts_v, par):
    """full [256, padn] rhs (float64) for a group/parity."""
    for g in _GROUPS:
        if g[0] == gname:
            _, _, lvl, opairs, realn, padn, _ = g
            break
    vs = consts_v[gname]
    parts = [_pair_rhs(V, par) for V in vs]
    rhs = np.concatenate(parts, axis=1)
    full = np.zeros((256, padn), dtype=_F64)
    full[:, :rhs.shape[1]] = rhs
    return full


def _make_vsrc(h0o, h1o, h0a, h0b, h1a, h1b):
    R0 = _conv_mat(h0o, 256)
    R1 = _conv_mat(h1o, 256)
    C0, C1 = R0, R1
    D0 = _dec_mat(h0b, h0a, False, 256)
    D1 = _dec_mat(h1b, h1a, True, 256)
    E0, E1 = D0, D1
    G0 = _dec_mat(h0b, h0a, False, 128)
    G1 = _dec_mat(h1b, h1a, True, 128)
    F0, F1 = G0, G1

    PE0 = C0 @ E0
    PE1 = C0 @ E1
    PE0F0 = PE0 @ F0
    PE0F1 = PE0 @ F1
    QD0 = R0 @ D0
    QD1 = R0 @ D1
    QD0G0 = QD0 @ G0
    QD0G1 = QD0 @ G1

    ucols = {
        "C0e": C0[:, 0::2], "C0o": C0[:, 1::2],
        "C1e": C1[:, 0::2], "C1o": C1[:, 1::2],
        "PE0e": PE0[:, 0::2], "PE0o": PE0[:, 1::2],
        "PE1e": PE1[:, 0::2], "PE1o": PE1[:, 1::2],
        "PF0e": PE0F0[:, 0::2], "PF0o": PE0F0[:, 1::2],
        "PF1e": PE0F1[:, 0::2], "PF1o": PE0F1[:, 1::2],
    }
    # V matrices per group (list => concatenated side by side)
    gv = {"lh1": [R0], "hh1": [R1], "hl1": [R1],
          "lhh2": [QD0, QD1], "hl2": [QD1],
          "lhh3": [QD0G0, QD0G1], "hl3": [QD0G1]}
    return ucols, gv, QD0G0


def _build_consts(h0o, h1o, h0a, h0b, h1a, h1b):
    ucols, gv, QD0G0 = _make_vsrc(h0o, h1o, h0a, h0b, h1a, h1b)

    U_all = np.zeros((256, _NU), dtype=_F64)
    for name, (off, nc_) in _UBLOCKS.items():
        M = ucols[name]
        assert M.shape == (256, nc_)
        U_all[:, off:off + nc_] = M

    blocks = [("u", U_all)]
    for g in _GROUPS:
        gname = g[0]
        for par in ("e", "o"):
            blocks.append((f"{gname}{par}", _group_rhs(gname, gv, par)))
    blocks.append(("ll3v", QD0G0))
    cc = np.concatenate([b[1] for b in blocks], axis=1)
    return {f"cc{k}": np.ascontiguousarray(
        cc[k * 128:(k + 1) * 128]).astype(np.float32) for k in range(2)}


def _const_offsets():
    """column offsets of each packed constant block inside cc{k}."""
    offs = {}
    off = 0
    for name, w in ([("u", _NU)]
                    + [(f"{g[0]}{par}", g[5]) for g in _GROUPS
                       for par in ("e", "o")]
                    + [("ll3v", 64)]):
        offs[name] = off
        off += w
    return offs, off


def _build_ranges():
    """Structural nonzero column ranges per (group, parity, kchunk), computed
    with all-ones filters (depends only on tap counts, not values).
    Pass 1 (e, k=0) is always streamed full-N with start=True."""
    ucols, gv, _ = _make_vsrc(np.ones(5), np.ones(7), np.ones(10),
                              np.ones(10), np.ones(10), np.ones(10))
    ranges = {}
    for g in _GROUPS:
        gname, padn = g[0], g[5]
        cost_full = padn * (1 if padn >= 256 else 4)
        for par in ("e", "o"):
            full = _group_rhs(gname, gv, par)
            for k in range(2):
                blk = full[k * 128:(k + 1) * 128]
                nz = np.nonzero(np.any(blk != 0.0, axis=0))[0]
                lo, hi = int(nz.min()), int(nz.max()) + 1
                w = hi - lo
                # float32r runs 4 cyc/row below N=256 — band-limiting only
                # pays off when the narrowed stream is still >=256 wide
                cost_rng = w * (1 if w >= 256 else 4)
                if cost_rng >= cost_full:
                    lo, hi = 0, padn
                ranges[(gname, par, k)] = (lo, hi)
    return ranges



# xc group-tile layout (4 channels, block-major): each U-block's 4 channel
# copies are adjacent so channel-packed lhsT slices are contiguous 1-D APs.
_XCOFF = {
    "C0e": (0, 128), "C0o": (512, 128), "C1e": (1024, 128), "C1o": (1536, 128),
    "PE0e": (2048, 64), "PE0o": (2304, 64), "PE1e": (2560, 64),
    "PE1o": (2816, 64),
    "PF0e": (3072, 32), "PF0o": (3200, 32), "PF1e": (3328, 32),
    "PF1o": (3456, 32),
}

# ---------------------------------------------------------------------------
# bass program (built once; constants are runtime inputs)
# ---------------------------------------------------------------------------

_CACHE = {}


def _build_program():
    import concourse.tile as tile
    from concourse import bacc, mybir
    from contextlib import ExitStack

    mmdt = getattr(mybir.dt, MM_DTYPE)
    f32 = mybir.dt.float32
    ranges = _build_ranges()

    nc = bacc.Bacc("TRN2", target_bir_lowering=False, debug=False)

    x_d = nc.dram_tensor("x", [CH_PER_CORE, 256, 256], mmdt,
                         kind="ExternalInput").ap()
    coffs, ctot = _const_offsets()
    const_d = {f"cc{k}": nc.dram_tensor(f"cc{k}", [128, ctot], mmdt,
                                        kind="ExternalInput").ap()
               for k in range(2)}

    yl_d = nc.dram_tensor("yl", [CH_PER_CORE, 64, 64], f32,
                          kind="ExternalOutput").ap()
    yh1_d = nc.dram_tensor("yh1", [CH_PER_CORE, 6, 128, 128, 2], f32,
                           kind="ExternalOutput").ap()
    # orientation-major so (channel, h) fuse into the DMA partition dim
    yh2_d = nc.dram_tensor("yh2", [6, CH_PER_CORE, 64, 64, 2], f32,
                           kind="ExternalOutput").ap()
    yh3_d = nc.dram_tensor("yh3", [6, CH_PER_CORE, 32, 32, 2], f32,
                           kind="ExternalOutput").ap()
    yh_d = {1: yh1_d, 2: yh2_d, 3: yh3_d}

    copy_ctr = [0]
    dma_ctr = [0]

    with tile.TileContext(nc) as tc:
        with ExitStack() as ctx:
            cpool = ctx.enter_context(tc.tile_pool(name="consts", bufs=1))
            xpool = ctx.enter_context(tc.tile_pool(name="xin", bufs=8))
            xcps_pool = ctx.enter_context(
                tc.tile_pool(name="xcps", bufs=4, space="PSUM"))
            xc_pool = ctx.enter_context(tc.tile_pool(name="xcsb", bufs=2))
            ps2_pool = ctx.enter_context(
                tc.tile_pool(name="ps2", bufs=4, space="PSUM"))
            stg_pool = ctx.enter_context(tc.tile_pool(name="stg", bufs=4))

            def copy_out(dst, src):
                if copy_ctr[0] % 2 == 0:
                    nc.vector.tensor_copy(dst, src)
                else:
                    nc.scalar.copy(dst, src)
                copy_ctr[0] += 1

            def dma(dst, src):
                if dma_ctr[0] % 2 == 0:
                    nc.sync.dma_start(dst, src)
                else:
                    nc.scalar.dma_start(dst, src)
                dma_ctr[0] += 1

            ccsb = []
            for k in range(2):
                t = cpool.tile([128, ctot], mmdt, tag=f"cc{k}",
                               name=f"cc{k}_sb")
                dma(t[:, 0:_NU], const_d[f"cc{k}"][:, 0:_NU])
                ccsb.append(t)
            for k in range(2):
                dma(ccsb[k][:, _NU:ctot], const_d[f"cc{k}"][:, _NU:ctot])

            def cslice(name, k, lo, hi):
                return ccsb[k][:, coffs[name] + lo:coffs[name] + hi]

            def phase_a(g0):
                """x load + stage 1 for the 4 channels of group g0."""
                xcg = [xc_pool.tile([128, 4 * _NU], mmdt, tag=f"xcg{m}",
                                    name=f"xcg{m}_{g0}")
                       for m in range(2)]
                for ci in range(4):
                    ch = g0 + ci
                    xin = xpool.tile([128, 512], mmdt, tag="xin",
                                     name=f"xin_{ch}")
                    # partition p holds x[ch, p, :] | x[ch, p+128, :]
                    dma(xin[:].rearrange("p (k w) -> p k w", k=2),
                        x_d[ch].rearrange("(k p) w -> p k w", k=2))
                    for m in range(2):
                        for half in range(2):
                            ps = xcps_pool.tile([128, 512], f32, tag="xcps")
                            nhalf = 512 if half == 0 else 448
                            for k in range(2):
                                nc.tensor.matmul(
                                    ps[:, 0:nhalf],
                                    lhsT=xin[:, k * 256 + m * 128:
                                             k * 256 + (m + 1) * 128],
                                    rhs=cslice("u", k, half * 512,
                                               half * 512 + nhalf),
                                    start=(k == 0), stop=(k == 1))
                            if half == 0:
                                # 4 L1 blocks of 128 -> block-major scatter
                                dst = xcg[m][:, 0:2048].rearrange(
                                    "p (b c j) -> p b c j", b=4, c=4)
                                copy_out(dst[:, :, ci, :], ps[:, :])
                            else:
                                # 4 blocks of 64, then 4 blocks of 32
                                dst = xcg[m][:, 2048:3072].rearrange(
                                    "p (b c j) -> p b c j", b=4, c=4)
                                copy_out(dst[:, :, ci, :], ps[:, 0:256])
                                dst = xcg[m][:, 3072:3584].rearrange(
                                    "p (b c j) -> p b c j", b=4, c=4)
                                copy_out(dst[:, :, ci, :], ps[:, 256:384])
                return xcg

            def phase_b(g0, xcg):
                def s2_matmuls(gname, upre, ps, ci0, pack, padn):
                    """4 accumulation passes (par x kchunk) into ps."""
                    first = True
                    for par in ("e", "o"):
                        for k in range(2):
                            if first:
                                lo, hi = 0, padn
                            else:
                                lo, hi = ranges[(gname, par, k)]
                                # PSUM write offsets must be 128-col aligned
                                lo = (lo // 128) * 128
                                w = hi - lo
                                if w * (1 if w >= 256 else 4) >= padn:
                                    lo, hi = 0, padn
                            base, w = _XCOFF[upre + par]
                            lhsT = xcg[k][:, base + ci0 * w:
                                          base + (ci0 + pack) * w]
                            nc.tensor.matmul(
                                ps[:, lo:hi],
                                lhsT=lhsT,
                                rhs=cslice(gname + par, k, lo, hi),
                                start=first,
                                stop=(par == "o" and k == 1))
                            first = False

                # ---- level 1: per channel ----
                for ci in range(4):
                    ch = g0 + ci
                    for gname, upre, lvl, opairs, realn, padn, pack in \
                            _GROUPS[:3]:
                        ps = ps2_pool.tile([128, padn], f32, tag="ps2")
                        s2_matmuls(gname, upre, ps, ci, 1, padn)
                        stg = stg_pool.tile(
                            [128, 512], f32, tag=f"stg_{gname}",
                            name=f"stg_{gname}_{ch}")
                        copy_out(stg[:, :], ps[:, :])
                        oA, oB = opairs[0]
                        dest = yh_d[1][ch, oA:oB + 1:oB - oA].rearrange(
                            "o h w r -> h o (w r)")
                        dma(dest,
                            stg[:].rearrange("p (o n) -> p o n", o=2))

                # ---- level 2: per channel pair ----
                for ci0 in (0, 2):
                    ch0 = g0 + ci0
                    for gname, upre, lvl, opairs, realn, padn, pack in \
                            _GROUPS[3:5]:
                        ps = ps2_pool.tile([128, padn], f32, tag="ps2")
                        s2_matmuls(gname, upre, ps, ci0, 2, padn)
                        stg = stg_pool.tile([128, realn], f32,
                                            tag=f"stg_{gname}")
                        copy_out(stg[:, :], ps[:, 0:realn])
                        npair = realn // len(opairs)
                        for pi, (oA, oB) in enumerate(opairs):
                            dest = yh_d[2][oA:oB + 1:oB - oA,
                                           ch0:ch0 + 2].rearrange(
                                "o c h w r -> (c h) o (w r)")
                            dma(dest,
                                stg[:, pi * npair:(pi + 1) * npair]
                                .rearrange("p (o n) -> p o n", o=2))

                # ---- level 3 + ll3: per channel quad ----
                for gname, upre, lvl, opairs, realn, padn, pack in \
                        _GROUPS[5:]:
                    ps = ps2_pool.tile([128, padn], f32, tag="ps2")
                    s2_matmuls(gname, upre, ps, 0, 4, padn)
                    stg = stg_pool.tile([128, realn], f32, tag=f"stg_{gname}")
                    copy_out(stg[:, :], ps[:, 0:realn])
                    npair = realn // len(opairs)
                    for pi, (oA, oB) in enumerate(opairs):
                        dest = yh_d[3][oA:oB + 1:oB - oA,
                                       g0:g0 + 4].rearrange(
                            "o c h w r -> (c h) o (w r)")
                        dma(dest,
                            stg[:, pi * npair:(pi + 1) * npair]
                            .rearrange("p (o n) -> p o n", o=2))

                for par in ("e", "o"):
                    base, w = _XCOFF["PF0" + par]
                    ps = ps2_pool.tile([128, 64], f32, tag="ps2")
                    for k in range(2):
                        nc.tensor.matmul(
                            ps[:, :],
                            lhsT=xcg[k][:, base:base + 4 * w],
                            rhs=cslice("ll3v", k, 0, 64),
                            start=(k == 0), stop=(k == 1))
                    stg = stg_pool.tile([128, 64], f32, tag="stgll")
                    copy_out(stg[:, :], ps[:, :])
                    p0 = 0 if par == "e" else 1
                    dest = yl_d[g0:g0 + 4, p0::2, :].rearrange(
                        "c h w -> (c h) w")
                    dma(dest, stg[:, :])

            # software-pipelined emission: stage-1 of group g+1 is issued
            # to the PE before stage-2 of group g, so the PE never idles
            # (and HAM never re-throttles) at group boundaries
            groups = list(range(0, CH_PER_CORE, 4))
            pend = []  # [(g0, xcg)]
            for g0 in groups:
                pend.append((g0, phase_a(g0)))
                if len(pend) == 2:
                    pg0, pxcg = pend.pop(0)
                    phase_b(pg0, pxcg)
            for pg0, pxcg in pend:
                phase_b(pg0, pxcg)

    nc.compile()
    return nc


def _get_program():
    if "nc" not in _CACHE:
        _CACHE["nc"] = _build_program()
    return _CACHE["nc"]


# ---------------------------------------------------------------------------
# public entry point
# ---------------------------------------------------------------------------

def kernel(x, h0o, h1o, h0a, h0b, h1a, h1b, _return_results=False,
           _trace=False):
    from concourse import bass_utils

    x = np.ascontiguousarray(np.asarray(x, dtype=np.float32))
    B, C, H, W = x.shape
    assert (B, C, H, W) == (4, 64, 256, 256)

    consts = _build_consts(
        np.asarray(h0o, np.float64), np.asarray(h1o, np.float64),
        np.asarray(h0a, np.float64), np.asarray(h0b, np.float64),
        np.asarray(h1a, np.float64), np.asarray(h1b, np.float64))

    nc = _get_program()

    xf = x.reshape(B * C, H, W)
    in_maps = []
    for core in range(N_CORES):
        m = {"x": np.ascontiguousarray(
            xf[core * CH_PER_CORE:(core + 1) * CH_PER_CORE])}
        m.update(consts)
        in_maps.append(m)

    r = bass_utils.run_bass_kernel_spmd(nc, in_maps,
                                        core_ids=list(range(N_CORES)),
                                        trace=_trace)

    yl = np.concatenate([r.results[i]["yl"] for i in range(N_CORES)], axis=0)
    yh1 = np.concatenate([r.results[i]["yh1"] for i in range(N_CORES)], axis=0)
    yh2 = np.concatenate([r.results[i]["yh2"] for i in range(N_CORES)],
                         axis=1).transpose(1, 0, 2, 3, 4)
    yh3 = np.concatenate([r.results[i]["yh3"] for i in range(N_CORES)],
                         axis=1).transpose(1, 0, 2, 3, 4)

    out = (yl.reshape(B, C, 64, 64),
           yh1.reshape(B, C, 6, 128, 128, 2),
           np.ascontiguousarray(yh2).reshape(B, C, 6, 64, 64, 2),
           np.ascontiguousarray(yh3).reshape(B, C, 6, 32, 32, 2))
    if _return_results:
        return out, r
    return out
